# revision 4
# baseline (speedup 1.0000x reference)
"""Trainium2 Bass kernel v2 for nn_MultiHeadAttention (B=1, S=4096, d=768, 12 heads).

Sharding (8 cores): 2 head-groups (6 heads / 384 ch) x 4 query-blocks (1024 rows).
All-bf16 data path (f32 PSUM accum), f32r only for tiny broadcast matmuls.

Design (per core), driven by ACT (softmax exp) saturation -- exp of 25.2M
scores/core through the 128-lane 1.2GHz ScalarE is the hard floor (~35us per
512q x 128ch pair):
  - pairs (qc, p) of 512 queries x 128 channels: (0,0),(1,0) ride the
    projection windows, then (0,1),(0,2),(1,1),(1,2)
  - per pair, 16 groups of GK=2 key-tiles, software-pipelined within the pair:
    scores (4 MMs, the 2 heads row-group concurrent on the PE) -> exp (one
    N=1024 ACT call per head, PSUM-direct) -> PV 2 groups behind (65-col
    serial MMs; ones-row gives the softmax denominator)
  - PSUM: psS h0/h1 scores slots (4 banks, exp-paced ring); psV: pvh tag
    (2 banks, PV accumulators + 1/l broadcast) + pp tag (2 banks, a dedicated
    projection ring so projections never serialize behind exp)
  - projections: weights + full KT resident in SBUF, kproj p-major so pair
    (0,0) starts right after kproj p=0; vproj rides pair (0,0)'s window,
    kproj p=1,2 ride pair (1,0)'s; oproj(qc=0) hides under pair (1,1).
Host: sums the 2 head-group partials per query block and adds bv@Wo + bo.
"""

import sys

sys.path.insert(0, "/opt/trn_rl_repo")

import numpy as np

import concourse.bass as bass
import concourse.mybir as mybir
from concourse.bass import ts, ds
from concourse.bass_utils import run_bass_kernel_spmd
from concourse.tile import TileContext

D_MODEL = 768
S = 4096
NH = 12
HD = 64
HG = 2
QB = 4
C = D_MODEL // HG       # 384 channels per head-group
NHL = NH // HG          # 6 heads per group
QR = S // QB            # 1024 query rows per block
NCORES = 8
SCALE = float(1.0 / np.sqrt(np.float32(D_MODEL)))
NKT = S // 128          # 32 key tiles
GK = 2                  # key tiles per group
NG = NKT // GK          # 16 groups per pair

F32 = mybir.dt.float32
F32R = mybir.dt.float32r
BF16 = mybir.dt.bfloat16
I16 = mybir.dt.int16
AF = mybir.ActivationFunctionType
OP = mybir.AluOpType

# Schraudolph exp in bf16 bits: bits = s * (2^7/ln2) + SCHR_B  (i16, bitcast bf16)
SCHR_A = 184.6650390625
SCHR_B = 16250.4            # HW-calibrated: centers the sawtooth error at +-3.3%


def _r(ap):
    return ap.bitcast(F32R)


def _split_excess_waits(nc, max_waits=1):
    """walrus rejects instructions carrying more than one semaphore wait."""
    n_split = 0
    for f in nc.m.functions:
        for blk in f.blocks:
            new_insts = []
            for inst in blk.instructions:
                si = inst.sync_info
                if si is not None and si.on_wait and len(si.on_wait) > max_waits:
                    waits = list(si.on_wait)
                    keep = waits[-max_waits:]
                    extra = waits[:-max_waits]
                    for i in range(0, len(extra), max_waits):
                        chunk = extra[i : i + max_waits]
                        nop = mybir.InstNoOp(
                            name=f"{inst.name}_wsplit_{i}",
                            ins=[],
                            outs=[],
                            engine=inst.engine,
                            sync_info=mybir.SyncInfo(on_wait=chunk, on_update=[]),
                        )
                        new_insts.append(nop)
                        n_split += 1
                    si.on_wait = keep
                new_insts.append(inst)
            blk.instructions = new_insts
    return n_split


PROF_LK = 256
PROF_TICK_CYC = 4800
PROF_NSNAP = 12


class _Prof:
    def __init__(self, nc, prog_ap, PROG):
        self.nc = nc
        self.prog_ap = prog_ap
        self.PROG = PROG

    def snap(self, idx, gate):
        from concourse.tile_rust import add_dep_helper
        d = self.nc.sync.dma_start(out=self.PROG[ds(idx, 1), :], in_=self.prog_ap)
        add_dep_helper(d.ins, gate.ins, sync=True, reason=f"prof snap {idx}")


def _emit_prof_ladder(nc, prog_ap):
    ladder = []
    reg_ctx = nc.gpsimd.register("prof_tick")
    reg = reg_ctx.__enter__()
    z = nc.gpsimd.reg_alu(reg, 0, 0, OP.add)
    ladder.append(z.ins)
    for i in range(PROF_LK):
        s = nc.gpsimd.store(prog_ap[0:1, ds(i, 1)], reg)
        ladder.append(s.ins)
    for i in range(PROF_LK):
        a = nc.gpsimd.reg_alu(reg, reg, 1, OP.add)
        ladder.append(a.ins)
        s = nc.gpsimd.store(prog_ap[0:1, ds(i, 1)], reg)
        ladder.append(s.ins)
        n = nc.gpsimd.nop(cycle_cnt=PROF_TICK_CYC, nofuse=True)
        ladder.append(n.ins)
    ladder_set = set(id(x) for x in ladder)
    f = nc.m.functions[0]
    for blk in f.blocks:
        blk.instructions = [x for x in blk.instructions if id(x) not in ladder_set]
    for blk in f.blocks:
        if blk.name.startswith("tile_context"):
            blk.instructions[0:0] = ladder
            return
    raise RuntimeError("profiler: no tile_context block found for tick ladder")


def _emit_body(nc, tc, io, schr16=0, prof=None, dbg=False, pvlag=2, esbufs=6):
    QT, KT, VT, WQ, WK, WV, WO, BQ, OUT = (
        io["QT"], io["KT"], io["VT"], io["WQ"], io["WK"], io["WV"], io["WO"],
        io["BQ"], io["OUT"],
    )
    SCHR_GRPS = {
        0: set(), 1: {8}, 2: {5, 11}, 3: {4, 9, 14}, 4: {3, 7, 11, 15},
        5: {2, 5, 8, 11, 14}, 6: {1, 4, 7, 10, 13, 15}, 8: {1, 3, 5, 7, 9, 11, 13, 15},
    }[schr16]

    consts = tc.alloc_tile_pool(name="consts", bufs=1)
    persist = tc.alloc_tile_pool(name="persist", bufs=1)
    esp = tc.alloc_tile_pool(name="esp", bufs=esbufs)
    attnp = tc.alloc_tile_pool(name="attnp", bufs=1)
    obp = tc.alloc_tile_pool(name="obp", bufs=2)
    smallp = tc.alloc_tile_pool(name="smallp", bufs=2)
    psS = tc.alloc_tile_pool(name="psS", bufs=1, space="PSUM")
    psV = tc.alloc_tile_pool(name="psV", bufs=2, space="PSUM")

    # ---- weights -> SBUF (bf16) ----
    wq_t, wk_t, wv_t = [], [], []
    for i in range(6):
        wq = consts.tile([128, C], BF16, tag=f"wq{i}", name=f"wq{i}")
        d0 = nc.sync.dma_start(out=wq, in_=WQ[ts(i, 128), :])
        if prof is not None and i == 0 and not dbg:
            prof.snap(0, d0)
        wq_t.append(wq)
    for i in range(6):
        wk = consts.tile([128, C], BF16, tag=f"wk{i}", name=f"wk{i}")
        nc.sync.dma_start(out=wk, in_=WK[ts(i, 128), :])
        wk_t.append(wk)
    for i in range(6):
        wv = consts.tile([128, C], BF16, tag=f"wv{i}", name=f"wv{i}")
        nc.sync.dma_start(out=wv, in_=WV[ts(i, 128), :])
        wv_t.append(wv)
    wo_t = []
    for p in range(3):
        wo = consts.tile([128, D_MODEL], BF16, tag=f"wo{p}", name=f"wo{p}")
        nc.sync.dma_start(out=wo, in_=WO[ts(p, 128), :])
        wo_t.append(wo)
    bq_t = []
    for p in range(3):
        bq = consts.tile([128, 1], F32, tag=f"bq{p}", name=f"bq{p}")
        nc.sync.dma_start(out=bq, in_=BQ[ts(p, 128)].rearrange("(p one) -> p one", one=1))
        bq_t.append(bq)
    ones64 = consts.tile([1, 64], F32, tag="ones64", name="ones64")
    nc.vector.memset(ones64, 1.0)
    ones_col = consts.tile([128, NHL], F32, tag="ones_col", name="ones_col")
    nc.vector.memset(ones_col, 1.0)

    # ---- persistent activations ----
    kT_t = [persist.tile([128, S], BF16, tag=f"kT{p}", name=f"kT{p}") for p in range(3)]
    qT_t = [persist.tile([128, QR], BF16, tag=f"qT{p}", name=f"qT{p}") for p in range(3)]
    vext_t = [persist.tile([128, NHL, HD + 1], BF16, tag=f"vx{j}", name=f"vx{j}")
              for j in range(NKT)]
    # inputs resident
    ktin = [persist.tile([128, S], BF16, tag=f"ktin{i}", name=f"ktin{i}") for i in range(6)]
    qin = [persist.tile([128, QR], BF16, tag=f"qin{i}", name=f"qin{i}") for i in range(6)]
    for i in range(6):
        nc.sync.dma_start(out=qin[i], in_=QT[ts(i, 128), :])
    for quarter in range(4):
        for i in range(6):
            nc.sync.dma_start(out=ktin[i][:, ds(quarter * 1024, 1024)],
                              in_=KT[ts(i, 128), ds(quarter * 1024, 1024)])
    vinp = tc.alloc_tile_pool(name="vinp", bufs=2)

    # psS slot allocator: alternate h0/h1 slots for projection chunks too
    slot_ctr = [0]

    def psS_slot():
        t = psS.tile([128, 2, 512], F32, tag=f"psS{slot_ctr[0] % 2}",
                     name=f"slot{slot_ctr[0]}")
        slot_ctr[0] += 1
        return t

    # ---------- projection units (psV "pv" ring transient slots) ----------
    def qproj_unit(p, half):
        sp = psV.tile([128, 512], F32, tag="pp", name=f"qp{p}_{half}")
        for i in range(6):
            nc.tensor.matmul(
                sp, lhsT=wq_t[i][:, ts(p, 128)],
                rhs=qin[i][:, ds(half * 512, 512)],
                start=(i == 0), stop=(i == 5),
            )
        return nc.vector.tensor_scalar(
            qT_t[p][:, ds(half * 512, 512)], sp, SCALE, bq_t[p],
            OP.mult, OP.add)

    def kproj_unit(p, c8):
        sp = psV.tile([128, 512], F32, tag="pp", name=f"kp{p}_{c8}")
        for i in range(6):
            nc.tensor.matmul(
                sp, lhsT=wk_t[i][:, ts(p, 128)],
                rhs=ktin[i][:, ds(c8 * 512, 512)],
                start=(i == 0), stop=(i == 5),
            )
        return nc.vector.tensor_copy(kT_t[p][:, ds(c8 * 512, 512)], sp)

    def vproj_unit(vin, ck, jj):
        # key tile j = ck*8 + jj
        j = ck * 8 + jj
        sp = psV.tile([128, 512], F32, tag="pp", name=f"vp{j}")
        for i in range(6):
            nc.tensor.matmul(
                sp[:, 0:C], lhsT=vin[i][:, ds(jj * 128, 128)],
                rhs=wv_t[i], start=(i == 0), stop=(i == 5),
            )
        nc.vector.tensor_copy(
            vext_t[j][:, :, 0:HD], sp[:, 0:C].rearrange("p (h d) -> p h d", h=NHL))
        return nc.vector.tensor_copy(vext_t[j][:, :, HD], ones_col)

    # ---------- attention pairs ----------
    class Pair:
        def __init__(self, qc, p):
            self.qc, self.p = qc, p
            # Schraudolph only in the ACT-bound windows (not pairs (0,0)/(1,0),
            # whose windows are PE/DMA-bound and leave ACT slack)
            self.use_schr = not (p == 0)
            self.es = [None] * NG
            self.pvh = None
            self.at = attnp.tile([128, 512], BF16, tag=f"at{qc}_{p}",
                                 name=f"at{qc}_{p}")

        def scores_grp(self, g):
            for h in range(2):
                sp = psS_slot()
                for kt in range(GK):
                    j = g * GK + kt
                    nc.tensor.matmul(
                        sp[:, kt, :],
                        lhsT=kT_t[self.p][ds(64 * h, 64), ts(j, 128)],
                        rhs=qT_t[self.p][ds(64 * h, 64), ds(self.qc * 512, 512)],
                        start=True, stop=True,
                    )
                es = esp.tile([128, GK, 512], I16, tag="es",
                              name=f"es{self.qc}_{self.p}_{g}_{h}")
                if self.use_schr and g in SCHR_GRPS:
                    ei = nc.vector.tensor_scalar(es, sp, SCHR_A, SCHR_B, OP.mult, OP.add)
                else:
                    ei = nc.scalar.activation(es[:, :, :].bitcast(BF16), sp, AF.Exp)
                if dbg and prof is not None and (self.qc, self.p) == (0, 2) \
                        and h == 1 and g % 4 == 3:
                    prof.snap(g // 4, ei)
                self.es[g] = self.es[g] or [None, None]
                self.es[g][h] = es

        def pv_grp(self, g):
            # pairs riding the projection windows use serial 65-col PV (2
            # accumulators, pp slots busy with projections); pure pairs split
            # the contraction in rg-halves (v2 scheme) with the B-half
            # accumulators in the then-idle pp slots.
            split = self.p != 0
            if self.pvh is None:
                if split:
                    self.pvh = [[psV.tile([HD + 1, 512], F32, tag=tag,
                                          name=f"pv{self.qc}_{self.p}_{h}_{tag}")
                                 for tag in ("pvh", "pp")] for h in range(2)]
                else:
                    self.pvh = [psV.tile([HD + 1, 512], F32, tag="pvh",
                                         name=f"pv{self.qc}_{self.p}_{h}")
                                for h in range(2)]
            for h in range(2):
                es = self.es[g][h]
                head = 2 * self.p + h
                for kt in range(GK):
                    j = g * GK + kt
                    if split:
                        for half in range(2):
                            nc.tensor.matmul(
                                self.pvh[h][half],
                                lhsT=vext_t[j][ds(64 * half, 64), head, :],
                                rhs=es[ds(64 * half, 64), kt, :].bitcast(BF16),
                                start=(j == 0), stop=(j == NKT - 1),
                            )
                    else:
                        nc.tensor.matmul(
                            self.pvh[h],
                            lhsT=vext_t[j][:, head, :],
                            rhs=es[:, kt, :].bitcast(BF16),
                            start=(j == 0), stop=(j == NKT - 1),
                        )
            self.es[g] = None

        def end(self, snap_idx=None):
            gate = None
            split = self.p != 0
            for h in range(2):
                if split:
                    A, B = self.pvh[h]
                else:
                    A, B = self.pvh[h], None
                a_sb = smallp.tile([HD + 1, 512], F32, tag="asb",
                                   name=f"asb{self.qc}{self.p}{h}", bufs=2)
                nc.vector.tensor_copy(a_sb, A)
                if B is not None:
                    af = smallp.tile([HD + 1, 512], F32, tag="af",
                                     name=f"af{self.qc}{self.p}{h}", bufs=2)
                    nc.vector.tensor_tensor(af, a_sb, B, OP.add)
                else:
                    af = a_sb
                rr = smallp.tile([1, 512], F32, tag="rr", name=f"rr{self.qc}{self.p}{h}")
                with nc.allow_low_precision("1/l feeds f32r broadcast matmul"):
                    nc.vector.reciprocal(_r(rr), af[ds(HD, 1), :])
                rbc = psV.tile([64, 512], F32, tag="pvh", name=f"rbc{self.qc}{self.p}{h}")
                nc.tensor.matmul(rbc, lhsT=_r(ones64), rhs=_r(rr), start=True, stop=True)
                gate = nc.vector.tensor_mul(self.at[ds(64 * h, 64), :],
                                            af[ds(0, HD), :], rbc)
            if prof is not None and snap_idx is not None:
                prof.snap(snap_idx, gate)
            self.pvh = None

    pairs = {}
    for qc in range(2):
        for p in range(3):
            pairs[(qc, p)] = Pair(qc, p)

    def oproj_unit(qc, qs, ob):
        for oc in range(2):
            po = psS.tile([128, 2, 512], F32, tag=f"psS{slot_ctr[0] % 2}",
                          name=f"po{qc}_{qs}_{oc}")
            slot_ctr[0] += 1
            for p in range(3):
                nc.tensor.matmul(
                    po[:, 0, 0:384],
                    lhsT=pairs[(qc, p)].at[:, ts(qs, 128)],
                    rhs=wo_t[p][:, ts(oc, 384)],
                    start=(p == 0), stop=(p == 2),
                )
            nc.vector.tensor_copy(ob[:, ts(oc, 384)], po[:, 0, 0:384])
        return nc.sync.dma_start(out=OUT[ds(qc * 512 + qs * 128, 128), :], in_=ob)

    # ================= weave =================
    # Within-pair pipeline: PV lags scores by PVLAG groups; vproj rides in
    # pair (0,0)'s window, kproj p=1,2 in pair (1,0)'s.
    PVLAG = pvlag
    P00, P10 = pairs[(0, 0)], pairs[(1, 0)]

    # W0: qproj + kproj p=0
    qgate = None
    for p in range(3):
        for half in range(2):
            qgate = qproj_unit(p, half)
    if prof is not None and not dbg:
        prof.snap(1, qgate)
    for c8 in range(8):
        kproj_unit(0, c8)

    # W1: vproj || pair (0,0)
    vgate = None
    vin = None
    for j in range(NKT):
        ck = j // 8
        if j % 8 == 0:
            vin = [vinp.tile([128, 1024], BF16, tag=f"vin{i}", name=f"vin{i}_{ck}")
                   for i in range(6)]
            for i in range(6):
                nc.sync.dma_start(out=vin[i], in_=VT[ts(i, 128), ds(ck * 1024, 1024)])
        vgate = vproj_unit(vin, ck, j % 8)
        if j % 2 == 1:
            g = j // 2
            P00.scores_grp(g)
            if g >= PVLAG:
                P00.pv_grp(g - PVLAG)
    if prof is not None and not dbg:
        prof.snap(3, vgate)
    for g in range(NG - PVLAG, NG):
        P00.pv_grp(g)
    P00.end(snap_idx=4)

    # W2: kproj p=1,2 || pair (1,0)
    kgate = None
    for u in range(16):
        p_ = 1 + u // 8
        kgate = kproj_unit(p_, u % 8)
        g2 = u
        P10.scores_grp(g2)
        if g2 >= PVLAG:
            P10.pv_grp(g2 - PVLAG)
    if prof is not None and not dbg:
        prof.snap(2, kgate)
    for g in range(NG - PVLAG, NG):
        P10.pv_grp(g)
    P10.end(snap_idx=5)

    # W3..W6: pure pairs ((0,*) first so oproj(0) can hide under (1,1));
    # the next pair's first PVLAG score groups are emitted before the current
    # pair's PV tail so ACT never idles across the boundary
    seq = [(0, 1), (0, 2), (1, 1), (1, 2)]
    pre_done = set()
    for idx, cur in enumerate(seq):
        cp = pairs[cur]
        g0 = PVLAG if cur in pre_done else 0
        for g in range(g0, NG):
            cp.scores_grp(g)
            if g >= PVLAG:
                cp.pv_grp(g - PVLAG)
            if cur == (1, 1) and g % 4 == 3:
                qs = g // 4
                ob = obp.tile([128, D_MODEL], F32, tag="ob", name=f"ob0_{qs}")
                d = oproj_unit(0, qs, ob)
                if prof is not None and qs == 3 and not dbg:
                    prof.snap(10, d)
        if idx + 1 < len(seq):
            nxt = seq[idx + 1]
            for g2 in range(PVLAG):
                pairs[nxt].scores_grp(g2)
            pre_done.add(nxt)
        for g in range(NG - PVLAG, NG):
            cp.pv_grp(g)
        cp.end(snap_idx=6 + idx)
    for qs in range(4):
        ob = obp.tile([128, D_MODEL], F32, tag="ob", name=f"ob1_{qs}")
        d = oproj_unit(1, qs, ob)
        if prof is not None and qs == 3 and not dbg:
            prof.snap(11, d)

    for pool in [vinp, psV, psS, smallp, obp, attnp, esp, persist, consts]:
        pool.release()


_nc_cache = {}


def build_nc(schr16=0, split_waits=True, timing_mode=False, profile_ladder=False,
             dbg=False, pvlag=2, esbufs=6):
    key = (schr16, split_waits, timing_mode, profile_ladder, dbg, pvlag, esbufs)
    if key in _nc_cache:
        return _nc_cache[key]
    nc = bass.Bass()
    if timing_mode:
        nc.declare_dram_parameter("DUMMY", [1, 128], F32, isOutput=False)
        io = {
            "QT": nc.dram_tensor("QT", [D_MODEL, QR], BF16),
            "KT": nc.dram_tensor("KT", [D_MODEL, S], BF16),
            "VT": nc.dram_tensor("VT", [D_MODEL, S], BF16),
            "WQ": nc.dram_tensor("WQ", [D_MODEL, C], BF16),
            "WK": nc.dram_tensor("WK", [D_MODEL, C], BF16),
            "WV": nc.dram_tensor("WV", [D_MODEL, C], BF16),
            "WO": nc.dram_tensor("WO", [C, D_MODEL], BF16),
            "BQ": nc.dram_tensor("BQ", [C], F32),
            "OUT": nc.declare_dram_parameter("OUT", [QR, D_MODEL], F32, isOutput=True),
        }
    else:
        io = {
            "QT": nc.declare_dram_parameter("QT", [D_MODEL, QR], BF16, isOutput=False),
            "KT": nc.declare_dram_parameter("KT", [D_MODEL, S], BF16, isOutput=False),
            "VT": nc.declare_dram_parameter("VT", [D_MODEL, S], BF16, isOutput=False),
            "WQ": nc.declare_dram_parameter("WQ", [D_MODEL, C], BF16, isOutput=False),
            "WK": nc.declare_dram_parameter("WK", [D_MODEL, C], BF16, isOutput=False),
            "WV": nc.declare_dram_parameter("WV", [D_MODEL, C], BF16, isOutput=False),
            "WO": nc.declare_dram_parameter("WO", [C, D_MODEL], BF16, isOutput=False),
            "BQ": nc.declare_dram_parameter("BQ", [C], F32, isOutput=False),
            "OUT": nc.declare_dram_parameter("OUT", [QR, D_MODEL], F32, isOutput=True),
        }
    prof = None
    prog_ap = None
    if profile_ladder:
        PROG = nc.declare_dram_parameter(
            "PROG", [PROF_NSNAP, PROF_LK], mybir.dt.int32, isOutput=True)
        prog_ap = nc.alloc_sbuf_tensor("prog_ticks", [1, PROF_LK], mybir.dt.int32).ap()
        prof = _Prof(nc, prog_ap, PROG)
    with TileContext(nc) as tc:
        _emit_body(nc, tc, io, schr16=schr16, prof=prof, dbg=dbg, pvlag=pvlag,
                   esbufs=esbufs)
    if profile_ladder:
        _emit_prof_ladder(nc, prog_ap)
    if split_waits:
        _split_excess_waits(nc)
    _nc_cache[key] = nc
    return nc


def make_in_maps(Q, K, V, Wq, bq, Wk, bk, Wv, bv, Wo, bo):
    import ml_dtypes
    BF = ml_dtypes.bfloat16
    Qm = np.asarray(Q, np.float32).reshape(S, D_MODEL)
    Km = np.asarray(K, np.float32).reshape(S, D_MODEL)
    Vm = np.asarray(V, np.float32).reshape(S, D_MODEL)
    QTf = np.ascontiguousarray(Qm.T).astype(BF)
    KTf = np.ascontiguousarray(Km.T).astype(BF)
    VTf = np.ascontiguousarray(Vm.T).astype(BF)
    Wq = np.asarray(Wq, np.float32); Wk = np.asarray(Wk, np.float32)
    Wv = np.asarray(Wv, np.float32); Wo = np.asarray(Wo, np.float32)
    bq = np.asarray(bq, np.float32); bv = np.asarray(bv, np.float32)
    bo = np.asarray(bo, np.float32)

    in_maps = []
    for c in range(NCORES):
        g, b = divmod(c, QB)
        ch = slice(g * C, (g + 1) * C)
        in_maps.append({
            "QT": np.ascontiguousarray(QTf[:, b * QR : (b + 1) * QR]),
            "KT": KTf,
            "VT": VTf,
            "WQ": np.ascontiguousarray(Wq[:, ch]).astype(BF),
            "WK": np.ascontiguousarray(Wk[:, ch]).astype(BF),
            "WV": np.ascontiguousarray(Wv[:, ch]).astype(BF),
            "WO": np.ascontiguousarray(Wo[ch, :]).astype(BF),
            "BQ": np.ascontiguousarray(bq[ch] * np.float32(SCALE)),
        })
    host_const = (bv @ Wo + bo).astype(np.float32)
    return in_maps, host_const


def kernel(Q, K, V, Wq, bq, Wk, bk, Wv, bv, Wo, bo, schr16=0):
    nc = build_nc(schr16=schr16)
    in_maps, host_const = make_in_maps(Q, K, V, Wq, bq, Wk, bk, Wv, bv, Wo, bo)
    res = run_bass_kernel_spmd(nc, in_maps, core_ids=list(range(NCORES)))
    out = np.zeros((S, D_MODEL), np.float32)
    for c in range(NCORES):
        g, b = divmod(c, QB)
        out[b * QR : (b + 1) * QR, :] += res.results[c]["OUT"]
    out += host_const[None, :]
    return out.reshape(1, S, D_MODEL)
